# revision 38
# baseline (speedup 1.0000x reference)
"""GCN layer (gather + segment-sum + degree-normalize + linear) on 8 Trainium2 cores.

Strategy
--------
Destination-window sharding with load balancing: the 391 global windows of
128 dest nodes are LPT-assigned to 8 cores (49 slots each) by chunk count;
each core's windows are sorted descending so the shared SPMD per-slot chunk
template (max over cores of the j-th largest) stays tight (~1% padding vs
~9% for contiguous core slices).

Per 128-edge chunk, a PE matmul accumulates transposed aggregates
aggT[feat, dest] = G_chunk.T @ S_chunk in PSUM, where G is the dma_gather'd
source-feature chunk (bf16) and S[e, j] = (col_rel[e] == j) is built on DVE
in bf16 (fp8 S measured slower: mixed-dtype matmul takes a slow PE path).
The in-degree reciprocal is computed on the host (pure edge structure) and
shipped as a bf16 broadcast constant; per window hT = aggT * recip (DVE,
bf16); the linear epilogue is batched 4 windows per PE matmul (one wt
LDWEIGHTS per 512 output cols — one PSUM bank, the ISA cap) + bias
(per-partition scalar), DMA out bf16 to a transposed [out_f, dests] output
that the host de-transposes and casts to f32. No scatter-add, no
collectives.

Known limits (measured): the device duty-cycle power-throttles to ~50%
after ~94us, so gather stream (~230us, paced by Pool desc-gen at
~2.25ns/desc across 4 SWDGE queues) and PE (~230us incl. throttle) are
co-critical; startup ~19us is Q7 ucode LOAD_LIB + preamble.

Why not dma_scatter_add per edge: measured on HW, its read-modify-write races
lose updates whenever a destination index appears more than once per call.

dma_gather facts (measured): idx arrays are int16, wrapped [16, N/16] and
replicated into all eight 16-partition groups; single_packet=False is required
for calls over 1024 indices. int16 limits a gather call's index range to
32768 rows, so edges are split into lo/hi source streams gathered from base
x[0] / x[32768].
"""
import sys
import os
sys.path.insert(0, "/opt/trn_rl_repo")

import numpy as np

P = 128
GATHER_SPLIT = 32768       # max rows addressable by a signed-int16 gather index
DEFAULT_BLK_CHUNKS = 64    # gather call size in 128-edge chunks (8192 idxs)
SBATCH = 8                 # S-matrix build batch, in chunks
N_CORES = 8


def _ceil_div(a, b):
    return -(-a // b)


def _wrap_idx(ix):
    """[N] int16 -> [128, N/16], idx i at [i%16, i//16], replicated into the
    eight 16-partition groups (the tx/rx Q7 cpus of every SWDGE queue each
    read their own group)."""
    n = len(ix)
    assert n % 16 == 0
    w = np.zeros((P, n // 16), np.int16)
    blk = ix.reshape(-1, 16).T
    for g in range(8):
        w[16 * g:16 * (g + 1), :] = blk
    return w


class Plan:
    """Host-side sharding: per-core per-stream edge arrays with a chunk
    structure (slots x chunk counts) identical across cores, so a single
    SPMD program serves all cores."""

    def __init__(self, row, col, n_nodes, n_cores=N_CORES,
                 blk_chunks=DEFAULT_BLK_CHUNKS, gather_split=GATHER_SPLIT):
        self.n_cores = n_cores
        self.n_nodes = n_nodes
        self.gw = _ceil_div(n_nodes, P)          # global 128-dest windows
        self.n_win = _ceil_div(self.gw, n_cores)  # slots per core
        W = self.n_win
        self.blk_chunks = blk_chunks
        self.gather_split = gather_split

        order = np.argsort(col, kind="stable")
        rs = row[order]
        cs = col[order]
        wb = np.searchsorted(cs, np.arange(self.gw + 1) * P)  # window bounds

        # per-global-window stream edge lists and chunk counts
        win_edges = []  # [g] = {stream: (rows, col_rel)}
        c_lo = np.zeros(self.gw, np.int64)
        c_hi = np.zeros(self.gw, np.int64)
        for g in range(self.gw):
            a, b = wb[g], wb[g + 1]
            r_g = rs[a:b]
            cr_g = (cs[a:b] - g * P).astype(np.int64)
            lo = r_g < gather_split
            win_edges.append({"lo": (r_g[lo], cr_g[lo]),
                              "hi": (r_g[~lo], cr_g[~lo])})
            c_lo[g] = _ceil_div(len(r_g[lo]), P)
            c_hi[g] = _ceil_div(len(r_g[~lo]), P)

        # LPT assignment of windows to cores (minimize per-core chunk totals)
        tot = c_lo + c_hi
        self.core_windows = [[] for _ in range(n_cores)]
        load = np.zeros(n_cores, np.int64)
        for g in np.argsort(-tot, kind="stable"):
            k = min((kk for kk in range(n_cores)
                     if len(self.core_windows[kk]) < W),
                    key=lambda kk: (load[kk], len(self.core_windows[kk])))
            self.core_windows[k].append(int(g))
            load[k] += tot[g]
        for k in range(n_cores):  # slot-sort so per-stream slot maxes stay tight
            self.core_windows[k].sort(key=lambda g: (-c_lo[g], -c_hi[g]))
            self.core_windows[k] += [-1] * (W - len(self.core_windows[k]))

        # shared per-slot chunk template = max over cores
        cnt = {"lo": np.zeros(W, np.int64), "hi": np.zeros(W, np.int64)}
        for k in range(n_cores):
            for j, g in enumerate(self.core_windows[k]):
                if g < 0:
                    continue
                cnt["lo"][j] = max(cnt["lo"][j], c_lo[g])
                cnt["hi"][j] = max(cnt["hi"][j], c_hi[g])
        cnt["lo"] = np.maximum(cnt["lo"], 1)  # every slot gets >=1 chunk
        self.cnt = cnt
        self.off = {s: np.concatenate([[0], np.cumsum(cnt[s])]) for s in cnt}
        self.Csum = {s: int(self.off[s][-1]) for s in cnt}
        self.NB = {s: _ceil_div(self.Csum[s], blk_chunks) for s in cnt}
        self.Npad = {s: self.NB[s] * blk_chunks * P for s in cnt}

        # in-degree reciprocal (edge structure only), broadcast per core:
        # [P, W*P] f32, column j*P+t = 1/deg of core window j's dest t
        deg = np.bincount(col, minlength=n_nodes).astype(np.float32)
        recip = 1.0 / np.maximum(deg, 1.0)

        self.core_arrays = []
        for k in range(n_cores):
            rb = np.ones(W * P, np.float32)
            for j, g in enumerate(self.core_windows[k]):
                if g < 0:
                    continue
                v = min(P, n_nodes - g * P)
                rb[j * P:j * P + v] = recip[g * P:g * P + v]
            import ml_dtypes
            arrs = {"recipb": np.ascontiguousarray(
                np.tile(rb[None, :], (P, 1)).astype(ml_dtypes.bfloat16))}
            for sname in ("lo", "hi"):
                base = 0 if sname == "lo" else gather_split
                off = self.off[sname]
                gidx = np.zeros(self.Npad[sname], np.int16)
                crel = np.full(self.Csum[sname] * P, -1, np.int8)
                last = 0
                for j, g in enumerate(self.core_windows[k]):
                    if g < 0:
                        continue
                    r_s, cr_s = win_edges[g][sname]
                    if not len(r_s):
                        continue
                    p0 = off[j] * P
                    gidx[p0:p0 + len(r_s)] = (r_s - base).astype(np.int16)
                    crel[p0:p0 + len(r_s)] = cr_s.astype(np.int8)
                    last = max(last, p0 + len(r_s))
                # NOTE: trailing -1 idxs would be stripped by the Q7 desc-gen
                # (fewer descriptors), but the un-written SBUF then holds
                # garbage that can be NaN, and the PE propagates NaN*0 from
                # the zero S rows. Pads must stay 0 (gather finite x[0]).
                arrs[f"gidx_{sname}"] = _wrap_idx(gidx)
                arrs[f"crel_{sname}"] = np.ascontiguousarray(
                    crel.reshape(self.Csum[sname], P).T)
            self.core_arrays.append(arrs)

    @property
    def total_chunks(self):
        return self.Csum["lo"] + self.Csum["hi"]


def _patch_swdge_lane_by_queue():
    """Pin each dma_gather's DMASW semaphore lane to its SWDGE queue number.

    Tile assigns DMASW lanes round-robin in scheduled order, which breaks when
    instructions on different queues (whose completions are only FIFO within a
    queue) share a lane. One lane per queue keeps per-lane completion in-order
    and lets gathers on the 4 queues run concurrently.
    """
    import concourse.tile_sem_assignment as tsa
    from concourse import mybir
    if getattr(tsa.TileClockTick, "_lane_by_queue_patch", False):
        return
    orig = tsa.TileClockTick._assign_tick

    def patched(self, inst):
        if isinstance(inst, mybir.InstDMAGatherAnt):
            if not hasattr(self, "_q_lane_ctr"):
                self._q_lane_ctr = {}
            q = inst.queue_num
            n = self._q_lane_ctr.get(q, 0)
            self._q_lane_ctr[q] = n + 1
            saved = self.next_sw_dma_idx
            # two lanes per queue: same-queue calls alternate lanes so a
            # call's desc-gen overlaps the previous call's DMA completion
            # (per-queue completion is FIFO, so lane ordering stays valid)
            self.next_sw_dma_idx = q * 2 + (n % 2)
            try:
                return orig(self, inst)
            finally:
                self.next_sw_dma_idx = saved
        return orig(self, inst)

    tsa.TileClockTick._assign_tick = patched
    tsa.TileClockTick._lane_by_queue_patch = True


def build_program(plan, in_f, out_f):
    """Emit the SPMD Bass program (shared by all cores)."""
    from concourse import bacc, mybir
    import concourse.tile as tile
    from contextlib import ExitStack

    _patch_swdge_lane_by_queue()
    skip_gather = os.environ.get("K_SKIP_GATHER") == "1"
    skip_compute = os.environ.get("K_SKIP_COMPUTE") == "1"

    f32 = mybir.dt.float32
    i16 = mybir.dt.int16
    i8 = mybir.dt.int8
    bf16 = mybir.dt.bfloat16

    W = plan.n_win
    BLK = plan.blk_chunks

    nc = bacc.Bacc("TRN2", target_bir_lowering=False, debug=False,
                   num_devices=plan.n_cores, num_swdge_queues=4)

    x_d = nc.dram_tensor("xb", [plan.n_nodes, in_f], bf16,
                         kind="ExternalInput")
    wt_d = nc.dram_tensor("wt", [in_f, out_f], bf16, kind="ExternalInput")
    bias_d = nc.dram_tensor("bias", [P, 1], f32, kind="ExternalInput")
    iota_d = nc.dram_tensor("iota", [P, P], bf16, kind="ExternalInput")
    recipb_d = nc.dram_tensor("recipb", [P, W * P], bf16,
                              kind="ExternalInput")
    gidx_d, crel_d = {}, {}
    for s in ("lo", "hi"):
        if plan.Csum[s] == 0:
            continue
        gidx_d[s] = nc.dram_tensor(f"gidx_{s}", [P, plan.Npad[s] // 16], i16,
                                   kind="ExternalInput")
        crel_d[s] = nc.dram_tensor(f"crel_{s}", [P, plan.Csum[s]], i8,
                                   kind="ExternalInput")
    # transposed output [out_f, dests]; host de-transposes and casts to f32
    out_d = nc.dram_tensor("out", [out_f, W * P], bf16, kind="ExternalOutput")

    x_base = {"lo": x_d[:], "hi": x_d[plan.gather_split:, :]}

    with tile.TileContext(nc) as tc, ExitStack() as ctx:
        cpool = ctx.enter_context(tc.tile_pool(name="const", bufs=1))
        gpool = {s: ctx.enter_context(tc.tile_pool(name=f"g_{s}", bufs=2))
                 for s in ("lo", "hi")}
        spool = {s: ctx.enter_context(tc.tile_pool(name=f"s_{s}", bufs=4))
                 for s in ("lo", "hi")}
        epool = ctx.enter_context(tc.tile_pool(name="epi", bufs=3))
        hgpool = ctx.enter_context(tc.tile_pool(name="hg", bufs=2))
        opool = ctx.enter_context(tc.tile_pool(name="outs", bufs=1))
        apool = ctx.enter_context(tc.tile_pool(name="psum_a", bufs=6,
                                               space="PSUM"))
        hpool = ctx.enter_context(tc.tile_pool(name="psum_h", bufs=2,
                                               space="PSUM"))

        # ---- constants (gather-feeding tables first: the first dma_gather
        # waits on gidx, so the big recipb load must not queue ahead; the
        # head slice covering block 0 loads separately so call 0 starts
        # without waiting for the full table) ----
        HCOLS = BLK * P // 16
        gidx_head, gidx_rest, crel_b = {}, {}, {}
        for s in ("lo", "hi"):
            if plan.Csum[s] == 0:
                continue
            ncols = plan.Npad[s] // 16
            h = cpool.tile([P, min(HCOLS, ncols)], i16, name=f"gidxh{s}")
            nc.sync.dma_start(out=h[:], in_=gidx_d[s][:, :min(HCOLS, ncols)])
            gidx_head[s] = h
        for s in ("lo", "hi"):
            if plan.Csum[s] == 0:
                continue
            cri = cpool.tile([P, plan.Csum[s]], i8, name=f"creli{s}")
            nc.sync.dma_start(out=cri[:], in_=crel_d[s][:])
            crb = cpool.tile([P, plan.Csum[s]], bf16, name=f"crelb{s}")
            nc.vector.tensor_copy(out=crb[:], in_=cri[:])
            crel_b[s] = crb
        for s in ("lo", "hi"):
            if plan.Csum[s] == 0:
                continue
            ncols = plan.Npad[s] // 16
            if ncols > HCOLS:
                r = cpool.tile([P, ncols - HCOLS], i16, name=f"gidxr{s}")
                nc.sync.dma_start(out=r[:], in_=gidx_d[s][:, HCOLS:])
                gidx_rest[s] = r

        def gidx_slice(s, c0, nch):
            a, b = c0 * P // 16, (c0 + nch) * P // 16
            if b <= HCOLS:
                return gidx_head[s][:, a:b]
            assert a >= HCOLS
            return gidx_rest[s][:, a - HCOLS:b - HCOLS]
        # epilogue-only constants ride the Scalar ring so the Sync ring
        # serves the gather-critical tables without queuing behind them
        iota_t = cpool.tile([P, P], bf16)
        nc.scalar.dma_start(out=iota_t[:], in_=iota_d[:])
        wt_t = cpool.tile([in_f, out_f], bf16)
        nc.scalar.dma_start(out=wt_t[:], in_=wt_d[:])
        bias_t = cpool.tile([P, 1], f32)
        nc.scalar.dma_start(out=bias_t[:], in_=bias_d[:])
        recipb_t = cpool.tile([P, W * P], bf16)
        nc.scalar.dma_start(out=recipb_t[:], in_=recipb_d[:])

        # ---- lazily-emitted gather blocks and S batches ----
        g_tiles = {}
        qctr = [0]

        def get_g(s, b):
            if (s, b) not in g_tiles:
                gt = gpool[s].tile([P, BLK * in_f], bf16, name=f"G{s}{b}",
                                   tag=f"G{s}{b % 2}")
                c0 = b * BLK
                nch = min(BLK, max(plan.Csum[s] - c0, 0))
                if skip_gather:
                    nc.vector.memset(gt[:], 0.0)
                elif nch > 0:
                    # first/last block split in two calls: a small first call
                    # starts the drain sooner; a smaller release quantum at
                    # the end of the stream shortens the compute tail
                    if b == 0 and nch > 8:
                        segs = [(0, 8), (8, nch - 8)]
                    elif b == plan.NB[s] - 1 and nch > BLK // 2:
                        segs = [(0, BLK // 2), (BLK // 2, nch - BLK // 2)]
                    else:
                        segs = [(0, nch)]
                    for o, nseg in segs:
                        nc.gpsimd.dma_gather(
                            gt[:, o * in_f:(o + nseg) * in_f]
                            .rearrange("p (c e) -> p c e", e=in_f),
                            x_base[s],
                            gidx_slice(s, c0 + o, nseg),
                            nseg * P,
                            nseg * P,
                            in_f,
                            single_packet=False,
                            queue_num=qctr[0] % 4,
                        )
                        qctr[0] += 1
                g_tiles[(s, b)] = gt
            return g_tiles[(s, b)]

        s_tiles = {}

        def get_s(s, sb):
            if (s, sb) not in s_tiles:
                st = spool[s].tile([P, SBATCH * P], bf16, name=f"S{s}{sb}",
                                   tag=f"S{s}")
                nb = min(SBATCH, plan.Csum[s] - sb * SBATCH)
                in0 = crel_b[s][:, sb * SBATCH:sb * SBATCH + nb] \
                    .to_broadcast([P, nb, P])
                in1 = iota_t[:][:, None, :].to_broadcast([P, nb, P])
                outv = st[:].rearrange("p (b j) -> p b j", j=P)[:, :nb, :]
                nc.vector.tensor_tensor(out=outv, in0=in0, in1=in1,
                                        op=mybir.AluOpType.is_equal)
                s_tiles[(s, sb)] = st
            return s_tiles[(s, sb)]

        # ---- pre-issue every gather call, interleaved by stream progress,
        # so the Pool engine always has ready calls on all 4 queues ----
        order = sorted(
            [(s, b) for s in ("lo", "hi") for b in range(plan.NB[s])],
            key=lambda sb: (sb[1] + 0.5) / plan.NB[sb[0]])
        for s, b in order:
            get_g(s, b)

        if skip_compute:
            # touch each G tile minimally so gathers aren't dead-code'd
            acc = epool.tile([P, 1], f32, tag="acc")
            nc.vector.memset(acc[:], 0.0)
            for (s_, b_), gt in g_tiles.items():
                nc.vector.tensor_tensor(
                    out=acc[:], in0=acc[:], in1=gt[:, :2].bitcast(f32),
                    op=mybir.AluOpType.add)
            nc.sync.dma_start(out=out_d[:1, :1], in_=acc[:1, :])

        # ---- main window-slot loop ----
        GRP = 4  # one PSUM bank (512 f32) caps the group matmul width
        hT_g = [None]
        for w in range(0 if skip_compute else W):
            chunks = []
            for s in ("lo", "hi"):
                chunks += [(s, c) for c in
                           range(plan.off[s][w], plan.off[s][w + 1])]
            # aggT[feat, dest] accumulated over the slot's chunks
            psum_agg = apool.tile([P, P], f32, tag="agg", name=f"agg{w}")
            n = len(chunks)
            for i, (s, c) in enumerate(chunks):
                b, slot = divmod(c, BLK)
                sb, ssub = divmod(c, SBATCH)
                gt = get_g(s, b)
                st = get_s(s, sb)
                nc.tensor.matmul(
                    out=psum_agg[:],
                    lhsT=gt[:, slot * in_f:(slot + 1) * in_f],
                    rhs=st[:, ssub * P:(ssub + 1) * P],
                    start=(i == 0), stop=(i == n - 1))

            # epilogue batched per GRP windows: one wt LDW + wide matmul
            gi, go = divmod(w, GRP)
            if go == 0:
                gw_n = min(GRP, W - gi * GRP)
                hT_g[0] = hgpool.tile([P, gw_n * P], bf16, tag="h",
                                      name=f"h{gi}")
            nc.vector.tensor_tensor(
                out=hT_g[0][:, go * P:(go + 1) * P], in0=psum_agg[:],
                in1=recipb_t[:, w * P:(w + 1) * P],
                op=mybir.AluOpType.mult)
            if go == min(GRP, W - gi * GRP) - 1:
                gw_n = go + 1
                out_p = hpool.tile([out_f, gw_n * P], f32, tag="outp",
                                   name=f"outp{gi}")
                nc.tensor.matmul(out=out_p[:], lhsT=wt_t[:], rhs=hT_g[0][:],
                                 start=True, stop=True)
                out_t = opool.tile([out_f, gw_n * P], bf16, tag="outs",
                                   name=f"outs{gi}")
                nc.any.tensor_scalar_add(out=out_t[:], in0=out_p[:],
                                         scalar1=bias_t[:])
                nc.sync.dma_start(
                    out=out_d[:, gi * GRP * P:gi * GRP * P + gw_n * P],
                    in_=out_t[:])

    # Strip Pool-engine waits on DMASW lane sems: they serialize each lane's
    # desc-gen behind the previous same-lane DMA *completion*. Per-queue SWDGE
    # completion is FIFO in hardware, and G-tile slot WAW safety already flows
    # through the PE consumer waits (a reused slot's new gather waits on the
    # previous tile's readers, which waited on its DMASW sem). Consumers keep
    # their DMASW waits.
    for blk in nc.m.functions[0].blocks:
        for ins in blk.instructions:
            if ins.engine != mybir.EngineType.Pool:
                continue
            if not isinstance(ins, (mybir.InstDMAGatherAnt,
                                    mybir.InstEventSemaphore)):
                continue
            si = ins.sync_info
            if si is None or not si.on_wait:
                continue
            si.on_wait = [w for w in si.on_wait
                          if not (w.ant_name or "").startswith("DMASW")]

    nc.compile()
    return nc


def make_in_maps(plan, x, W, b):
    import ml_dtypes
    xf = np.ascontiguousarray(x, dtype=np.float32)
    base = {
        "xb": np.ascontiguousarray(xf.astype(ml_dtypes.bfloat16)),
        "wt": np.ascontiguousarray(W.T.astype(ml_dtypes.bfloat16)),
        "bias": np.asarray(b, np.float32).reshape(P, 1),
        "iota": np.tile(np.arange(P, dtype=ml_dtypes.bfloat16)[None, :],
                        (P, 1)),
    }
    in_maps = []
    for k in range(plan.n_cores):
        m = dict(base)
        for name, arr in plan.core_arrays[k].items():
            if name.startswith(("gidx", "crel")):
                s = name.split("_")[1]
                if plan.Csum[s] == 0:
                    continue
            m[name] = arr
        in_maps.append(m)
    return in_maps


def run(x, edge_index, n_nodes, W, b, trace=False, trace_cores=None):
    from concourse.bass_utils import run_bass_kernel_spmd

    x = np.asarray(x)
    edge_index = np.asarray(edge_index)
    W = np.asarray(W)
    b = np.asarray(b)
    n_nodes = int(n_nodes)
    row = edge_index[0].astype(np.int64)
    col = edge_index[1].astype(np.int64)

    plan = Plan(row, col, n_nodes)
    nc = build_program(plan, x.shape[1], W.shape[0])
    in_maps = make_in_maps(plan, x, W, b)
    res = run_bass_kernel_spmd(nc, in_maps, core_ids=list(range(plan.n_cores)),
                               trace=trace, trace_cores=trace_cores)
    out_f = W.shape[0]
    out = np.empty((n_nodes, out_f), np.float32)
    for k in range(plan.n_cores):
        o_k = np.asarray(res.results[k]["out"], np.float32)  # [out_f, W*P]
        for j, g in enumerate(plan.core_windows[k]):
            if g < 0:
                continue
            v = min(P, n_nodes - g * P)
            out[g * P:g * P + v] = o_k[:, j * P:j * P + v].T
    return np.ascontiguousarray(out), res


def kernel(x, edge_index, n_nodes, W, b):
    out, _ = run(x, edge_index, n_nodes, W, b)
    return out


# revision 42
# speedup vs baseline: 1.4595x; 1.4595x over previous
"""GCN layer (gather + segment-sum + degree-normalize + linear) on 8 Trainium2 cores.

Strategy
--------
Destination-window sharding with load balancing: the 391 global windows of
128 dest nodes are LPT-assigned to 8 cores (49 slots each) by chunk count;
each core's windows are sorted descending so the shared SPMD per-slot chunk
template (max over cores of the j-th largest) stays tight (~1% padding vs
~9% for contiguous core slices).

Per 128-edge chunk, a PE matmul accumulates transposed aggregates
aggT[feat, dest] = G_chunk.T @ S_chunk in PSUM, where G is the dma_gather'd
source-feature chunk (bf16) and S[e, j] = (col_rel[e] == j) is built on DVE
in bf16 (fp8 S measured slower: mixed-dtype matmul takes a slow PE path).
The in-degree reciprocal is computed on the host (pure edge structure) and
shipped as a bf16 broadcast constant; per window hT = aggT * recip (DVE,
bf16); the linear epilogue is batched 4 windows per PE matmul (one wt
LDWEIGHTS per 512 output cols — one PSUM bank, the ISA cap) + bias
(per-partition scalar), DMA out bf16 to a transposed [out_f, dests] output
that the host de-transposes and casts to f32. No scatter-add, no
collectives.

Known limits (measured): the device duty-cycle power-throttles to ~50%
after ~94us, so gather stream (~230us, paced by Pool desc-gen at
~2.25ns/desc across 4 SWDGE queues) and PE (~230us incl. throttle) are
co-critical; startup ~19us is Q7 ucode LOAD_LIB + preamble.

Why not dma_scatter_add per edge: measured on HW, its read-modify-write races
lose updates whenever a destination index appears more than once per call.

dma_gather facts (measured): idx arrays are int16, wrapped [16, N/16] and
replicated into all eight 16-partition groups; single_packet=False is required
for calls over 1024 indices. int16 limits a gather call's index range to
32768 rows, so edges are split into lo/hi source streams gathered from base
x[0] / x[32768].
"""
import sys
import os
sys.path.insert(0, "/opt/trn_rl_repo")

import numpy as np

P = 128
GATHER_SPLIT = 32768       # max rows addressable by a signed-int16 gather index
DEFAULT_BLK_CHUNKS = 16    # gather call size in 128-edge chunks (2048 idxs);
                           # 8192-idx calls measured 40% slower (Q7 per-call
                           # pipeline), so bigger is NOT better
SBATCH = 8                 # S-matrix build batch, in chunks
N_CORES = 8


def _ceil_div(a, b):
    return -(-a // b)


def _wrap_idx(ix):
    """[N] int16 -> [128, N/16], idx i at [i%16, i//16], replicated into the
    eight 16-partition groups (the tx/rx Q7 cpus of every SWDGE queue each
    read their own group)."""
    n = len(ix)
    assert n % 16 == 0
    w = np.zeros((P, n // 16), np.int16)
    blk = ix.reshape(-1, 16).T
    for g in range(8):
        w[16 * g:16 * (g + 1), :] = blk
    return w


class Plan:
    """Host-side sharding: per-core per-stream edge arrays with a chunk
    structure (slots x chunk counts) identical across cores, so a single
    SPMD program serves all cores."""

    def __init__(self, row, col, n_nodes, n_cores=N_CORES,
                 blk_chunks=DEFAULT_BLK_CHUNKS, gather_split=GATHER_SPLIT):
        self.n_cores = n_cores
        self.n_nodes = n_nodes
        self.gw = _ceil_div(n_nodes, P)          # global 128-dest windows
        self.n_win = _ceil_div(self.gw, n_cores)  # slots per core
        W = self.n_win
        self.blk_chunks = blk_chunks
        self.gather_split = gather_split

        order = np.argsort(col, kind="stable")
        rs = row[order]
        cs = col[order]
        wb = np.searchsorted(cs, np.arange(self.gw + 1) * P)  # window bounds

        # per-global-window stream edge lists and chunk counts
        win_edges = []  # [g] = {stream: (rows, col_rel)}
        c_lo = np.zeros(self.gw, np.int64)
        c_hi = np.zeros(self.gw, np.int64)
        for g in range(self.gw):
            a, b = wb[g], wb[g + 1]
            r_g = rs[a:b]
            cr_g = (cs[a:b] - g * P).astype(np.int64)
            lo = r_g < gather_split
            win_edges.append({"lo": (r_g[lo], cr_g[lo]),
                              "hi": (r_g[~lo], cr_g[~lo])})
            c_lo[g] = _ceil_div(len(r_g[lo]), P)
            c_hi[g] = _ceil_div(len(r_g[~lo]), P)

        # LPT assignment of windows to cores (minimize per-core chunk totals)
        tot = c_lo + c_hi
        self.core_windows = [[] for _ in range(n_cores)]
        load = np.zeros(n_cores, np.int64)
        for g in np.argsort(-tot, kind="stable"):
            k = min((kk for kk in range(n_cores)
                     if len(self.core_windows[kk]) < W),
                    key=lambda kk: (load[kk], len(self.core_windows[kk])))
            self.core_windows[k].append(int(g))
            load[k] += tot[g]
        for k in range(n_cores):  # slot-sort so per-stream slot maxes stay tight
            self.core_windows[k].sort(key=lambda g: (-c_lo[g], -c_hi[g]))
            self.core_windows[k] += [-1] * (W - len(self.core_windows[k]))

        # shared per-slot chunk template = max over cores
        cnt = {"lo": np.zeros(W, np.int64), "hi": np.zeros(W, np.int64)}
        for k in range(n_cores):
            for j, g in enumerate(self.core_windows[k]):
                if g < 0:
                    continue
                cnt["lo"][j] = max(cnt["lo"][j], c_lo[g])
                cnt["hi"][j] = max(cnt["hi"][j], c_hi[g])
        cnt["lo"] = np.maximum(cnt["lo"], 1)  # every slot gets >=1 chunk
        self.cnt = cnt
        self.off = {s: np.concatenate([[0], np.cumsum(cnt[s])]) for s in cnt}
        self.Csum = {s: int(self.off[s][-1]) for s in cnt}
        self.NB = {s: _ceil_div(self.Csum[s], blk_chunks) for s in cnt}
        self.Npad = {s: self.NB[s] * blk_chunks * P for s in cnt}

        # in-degree reciprocal (edge structure only), broadcast per core:
        # [P, W*P] f32, column j*P+t = 1/deg of core window j's dest t
        deg = np.bincount(col, minlength=n_nodes).astype(np.float32)
        recip = 1.0 / np.maximum(deg, 1.0)

        self.core_arrays = []
        for k in range(n_cores):
            rb = np.ones(W * P, np.float32)
            for j, g in enumerate(self.core_windows[k]):
                if g < 0:
                    continue
                v = min(P, n_nodes - g * P)
                rb[j * P:j * P + v] = recip[g * P:g * P + v]
            import ml_dtypes
            arrs = {"recipb": np.ascontiguousarray(
                np.tile(rb[None, :], (P, 1)).astype(ml_dtypes.bfloat16))}
            for sname in ("lo", "hi"):
                base = 0 if sname == "lo" else gather_split
                off = self.off[sname]
                gidx = np.zeros(self.Npad[sname], np.int16)
                crel = np.full(self.Csum[sname] * P, -1, np.int8)
                last = 0
                for j, g in enumerate(self.core_windows[k]):
                    if g < 0:
                        continue
                    r_s, cr_s = win_edges[g][sname]
                    if not len(r_s):
                        continue
                    p0 = off[j] * P
                    gidx[p0:p0 + len(r_s)] = (r_s - base).astype(np.int16)
                    crel[p0:p0 + len(r_s)] = cr_s.astype(np.int8)
                    last = max(last, p0 + len(r_s))
                # NOTE: trailing -1 idxs would be stripped by the Q7 desc-gen
                # (fewer descriptors), but the un-written SBUF then holds
                # garbage that can be NaN, and the PE propagates NaN*0 from
                # the zero S rows. Pads must stay 0 (gather finite x[0]).
                arrs[f"gidx_{sname}"] = _wrap_idx(gidx)
                arrs[f"crel_{sname}"] = np.ascontiguousarray(
                    crel.reshape(self.Csum[sname], P).T)
            self.core_arrays.append(arrs)

    @property
    def total_chunks(self):
        return self.Csum["lo"] + self.Csum["hi"]


def _patch_swdge_lane_by_queue():
    """Pin each dma_gather's DMASW semaphore lane to its SWDGE queue number.

    Tile assigns DMASW lanes round-robin in scheduled order, which breaks when
    instructions on different queues (whose completions are only FIFO within a
    queue) share a lane. One lane per queue keeps per-lane completion in-order
    and lets gathers on the 4 queues run concurrently.
    """
    import concourse.tile_sem_assignment as tsa
    from concourse import mybir
    if getattr(tsa.TileClockTick, "_lane_by_queue_patch", False):
        return
    orig = tsa.TileClockTick._assign_tick

    def patched(self, inst):
        if isinstance(inst, mybir.InstDMAGatherAnt):
            if not hasattr(self, "_q_lane_ctr"):
                self._q_lane_ctr = {}
            q = inst.queue_num
            n = self._q_lane_ctr.get(q, 0)
            self._q_lane_ctr[q] = n + 1
            saved = self.next_sw_dma_idx
            # two lanes per queue: same-queue calls alternate lanes so a
            # call's desc-gen overlaps the previous call's DMA completion
            # (per-queue completion is FIFO, so lane ordering stays valid)
            self.next_sw_dma_idx = q * 2 + (n % 2)
            try:
                return orig(self, inst)
            finally:
                self.next_sw_dma_idx = saved
        return orig(self, inst)

    tsa.TileClockTick._assign_tick = patched
    tsa.TileClockTick._lane_by_queue_patch = True


def build_program(plan, in_f, out_f):
    """Emit the SPMD Bass program (shared by all cores)."""
    from concourse import bacc, mybir
    import concourse.tile as tile
    from contextlib import ExitStack

    _patch_swdge_lane_by_queue()
    skip_gather = os.environ.get("K_SKIP_GATHER") == "1"
    skip_compute = os.environ.get("K_SKIP_COMPUTE") == "1"

    f32 = mybir.dt.float32
    i16 = mybir.dt.int16
    i8 = mybir.dt.int8
    bf16 = mybir.dt.bfloat16

    W = plan.n_win
    BLK = plan.blk_chunks

    nc = bacc.Bacc("TRN2", target_bir_lowering=False, debug=False,
                   num_devices=plan.n_cores, num_swdge_queues=4)

    x_d = nc.dram_tensor("xb", [plan.n_nodes, in_f], bf16,
                         kind="ExternalInput")
    wt_d = nc.dram_tensor("wt", [in_f, out_f], bf16, kind="ExternalInput")
    bias_d = nc.dram_tensor("bias", [P, 1], f32, kind="ExternalInput")
    iota_d = nc.dram_tensor("iota", [P, P], bf16, kind="ExternalInput")
    recipb_d = nc.dram_tensor("recipb", [P, W * P], bf16,
                              kind="ExternalInput")
    gidx_d, crel_d = {}, {}
    for s in ("lo", "hi"):
        if plan.Csum[s] == 0:
            continue
        gidx_d[s] = nc.dram_tensor(f"gidx_{s}", [P, plan.Npad[s] // 16], i16,
                                   kind="ExternalInput")
        crel_d[s] = nc.dram_tensor(f"crel_{s}", [P, plan.Csum[s]], i8,
                                   kind="ExternalInput")
    # transposed output [out_f, dests]; host de-transposes and casts to f32
    out_d = nc.dram_tensor("out", [out_f, W * P], bf16, kind="ExternalOutput")

    x_base = {"lo": x_d[:], "hi": x_d[plan.gather_split:, :]}

    with tile.TileContext(nc) as tc, ExitStack() as ctx:
        cpool = ctx.enter_context(tc.tile_pool(name="const", bufs=1))
        gpool = {s: ctx.enter_context(tc.tile_pool(name=f"g_{s}", bufs=10))
                 for s in ("lo", "hi")}
        spool = {s: ctx.enter_context(tc.tile_pool(name=f"s_{s}", bufs=4))
                 for s in ("lo", "hi")}
        epool = ctx.enter_context(tc.tile_pool(name="epi", bufs=3))
        hgpool = ctx.enter_context(tc.tile_pool(name="hg", bufs=2))
        opool = ctx.enter_context(tc.tile_pool(name="outs", bufs=1))
        apool = ctx.enter_context(tc.tile_pool(name="psum_a", bufs=6,
                                               space="PSUM"))
        hpool = ctx.enter_context(tc.tile_pool(name="psum_h", bufs=2,
                                               space="PSUM"))

        # ---- constants (gather-feeding tables first: the first dma_gather
        # waits on gidx, so the big recipb load must not queue ahead; the
        # head slice covering block 0 loads separately so call 0 starts
        # without waiting for the full table) ----
        HCOLS = BLK * P // 16
        gidx_head, gidx_rest, crel_b = {}, {}, {}
        for s in ("lo", "hi"):
            if plan.Csum[s] == 0:
                continue
            ncols = plan.Npad[s] // 16
            h = cpool.tile([P, min(HCOLS, ncols)], i16, name=f"gidxh{s}")
            nc.sync.dma_start(out=h[:], in_=gidx_d[s][:, :min(HCOLS, ncols)])
            gidx_head[s] = h
        for s in ("lo", "hi"):
            if plan.Csum[s] == 0:
                continue
            cri = cpool.tile([P, plan.Csum[s]], i8, name=f"creli{s}")
            nc.sync.dma_start(out=cri[:], in_=crel_d[s][:])
            crb = cpool.tile([P, plan.Csum[s]], bf16, name=f"crelb{s}")
            nc.vector.tensor_copy(out=crb[:], in_=cri[:])
            crel_b[s] = crb
        for s in ("lo", "hi"):
            if plan.Csum[s] == 0:
                continue
            ncols = plan.Npad[s] // 16
            if ncols > HCOLS:
                r = cpool.tile([P, ncols - HCOLS], i16, name=f"gidxr{s}")
                nc.sync.dma_start(out=r[:], in_=gidx_d[s][:, HCOLS:])
                gidx_rest[s] = r

        def gidx_slice(s, c0, nch):
            a, b = c0 * P // 16, (c0 + nch) * P // 16
            if b <= HCOLS:
                return gidx_head[s][:, a:b]
            assert a >= HCOLS
            return gidx_rest[s][:, a - HCOLS:b - HCOLS]
        # epilogue-only constants ride the Scalar ring so the Sync ring
        # serves the gather-critical tables without queuing behind them
        iota_t = cpool.tile([P, P], bf16)
        nc.scalar.dma_start(out=iota_t[:], in_=iota_d[:])
        wt_t = cpool.tile([in_f, out_f], bf16)
        nc.scalar.dma_start(out=wt_t[:], in_=wt_d[:])
        bias_t = cpool.tile([P, 1], f32)
        nc.scalar.dma_start(out=bias_t[:], in_=bias_d[:])
        recipb_t = cpool.tile([P, W * P], bf16)
        nc.scalar.dma_start(out=recipb_t[:], in_=recipb_d[:])

        # ---- lazily-emitted gather blocks and S batches ----
        g_tiles = {}
        qctr = [0]

        def get_g(s, b):
            if (s, b) not in g_tiles:
                gt = gpool[s].tile([P, BLK * in_f], bf16, name=f"G{s}{b}",
                                   tag=f"G{s}{b % 2}")
                c0 = b * BLK
                nch = min(BLK, max(plan.Csum[s] - c0, 0))
                if skip_gather:
                    nc.vector.memset(gt[:], 0.0)
                elif nch > 0:
                    # first/last block split in two calls: a small first call
                    # starts the drain sooner; a smaller release quantum at
                    # the end of the stream shortens the compute tail
                    if b == 0 and nch > 8:
                        segs = [(0, 8), (8, nch - 8)]
                    elif b == plan.NB[s] - 1 and nch > BLK // 2:
                        segs = [(0, BLK // 2), (BLK // 2, nch - BLK // 2)]
                    else:
                        segs = [(0, nch)]
                    for o, nseg in segs:
                        nc.gpsimd.dma_gather(
                            gt[:, o * in_f:(o + nseg) * in_f]
                            .rearrange("p (c e) -> p c e", e=in_f),
                            x_base[s],
                            gidx_slice(s, c0 + o, nseg),
                            nseg * P,
                            nseg * P,
                            in_f,
                            single_packet=False,
                            queue_num=qctr[0] % 4,
                        )
                        qctr[0] += 1
                g_tiles[(s, b)] = gt
            return g_tiles[(s, b)]

        s_tiles = {}

        def get_s(s, sb):
            if (s, sb) not in s_tiles:
                st = spool[s].tile([P, SBATCH * P], bf16, name=f"S{s}{sb}",
                                   tag=f"S{s}")
                nb = min(SBATCH, plan.Csum[s] - sb * SBATCH)
                in0 = crel_b[s][:, sb * SBATCH:sb * SBATCH + nb] \
                    .to_broadcast([P, nb, P])
                in1 = iota_t[:][:, None, :].to_broadcast([P, nb, P])
                outv = st[:].rearrange("p (b j) -> p b j", j=P)[:, :nb, :]
                nc.vector.tensor_tensor(out=outv, in0=in0, in1=in1,
                                        op=mybir.AluOpType.is_equal)
                s_tiles[(s, sb)] = st
            return s_tiles[(s, sb)]

        # ---- pre-issue every gather call, interleaved by stream progress,
        # so the Pool engine always has ready calls on all 4 queues ----
        order = sorted(
            [(s, b) for s in ("lo", "hi") for b in range(plan.NB[s])],
            key=lambda sb: (sb[1] + 0.5) / plan.NB[sb[0]])
        for s, b in order:
            get_g(s, b)

        if skip_compute:
            # touch each G tile minimally so gathers aren't dead-code'd
            acc = epool.tile([P, 1], f32, tag="acc")
            nc.vector.memset(acc[:], 0.0)
            for (s_, b_), gt in g_tiles.items():
                nc.vector.tensor_tensor(
                    out=acc[:], in0=acc[:], in1=gt[:, :2].bitcast(f32),
                    op=mybir.AluOpType.add)
            nc.sync.dma_start(out=out_d[:1, :1], in_=acc[:1, :])

        # ---- main window-slot loop ----
        GRP = 4  # one PSUM bank (512 f32) caps the group matmul width
        hT_g = [None]
        for w in range(0 if skip_compute else W):
            chunks = []
            for s in ("lo", "hi"):
                chunks += [(s, c) for c in
                           range(plan.off[s][w], plan.off[s][w + 1])]
            # aggT[feat, dest] accumulated over the slot's chunks
            psum_agg = apool.tile([P, P], f32, tag="agg", name=f"agg{w}")
            n = len(chunks)
            for i, (s, c) in enumerate(chunks):
                b, slot = divmod(c, BLK)
                sb, ssub = divmod(c, SBATCH)
                gt = get_g(s, b)
                st = get_s(s, sb)
                nc.tensor.matmul(
                    out=psum_agg[:],
                    lhsT=gt[:, slot * in_f:(slot + 1) * in_f],
                    rhs=st[:, ssub * P:(ssub + 1) * P],
                    start=(i == 0), stop=(i == n - 1))

            # epilogue batched per GRP windows: one wt LDW + wide matmul
            gi, go = divmod(w, GRP)
            if go == 0:
                gw_n = min(GRP, W - gi * GRP)
                hT_g[0] = hgpool.tile([P, gw_n * P], bf16, tag="h",
                                      name=f"h{gi}")
            nc.vector.tensor_tensor(
                out=hT_g[0][:, go * P:(go + 1) * P], in0=psum_agg[:],
                in1=recipb_t[:, w * P:(w + 1) * P],
                op=mybir.AluOpType.mult)
            if go == min(GRP, W - gi * GRP) - 1:
                gw_n = go + 1
                out_p = hpool.tile([out_f, gw_n * P], f32, tag="outp",
                                   name=f"outp{gi}")
                nc.tensor.matmul(out=out_p[:], lhsT=wt_t[:], rhs=hT_g[0][:],
                                 start=True, stop=True)
                out_t = opool.tile([out_f, gw_n * P], bf16, tag="outs",
                                   name=f"outs{gi}")
                nc.any.tensor_scalar_add(out=out_t[:], in0=out_p[:],
                                         scalar1=bias_t[:])
                nc.sync.dma_start(
                    out=out_d[:, gi * GRP * P:gi * GRP * P + gw_n * P],
                    in_=out_t[:])

    # Strip Pool-engine waits on DMASW lane sems: they serialize each lane's
    # desc-gen behind the previous same-lane DMA *completion*. Per-queue SWDGE
    # completion is FIFO in hardware, and G-tile slot WAW safety already flows
    # through the PE consumer waits (a reused slot's new gather waits on the
    # previous tile's readers, which waited on its DMASW sem). Consumers keep
    # their DMASW waits.
    for blk in nc.m.functions[0].blocks:
        for ins in blk.instructions:
            if ins.engine != mybir.EngineType.Pool:
                continue
            if not isinstance(ins, (mybir.InstDMAGatherAnt,
                                    mybir.InstEventSemaphore)):
                continue
            si = ins.sync_info
            if si is None or not si.on_wait:
                continue
            si.on_wait = [w for w in si.on_wait
                          if not (w.ant_name or "").startswith("DMASW")]

    nc.compile()
    return nc


def make_in_maps(plan, x, W, b):
    import ml_dtypes
    xf = np.ascontiguousarray(x, dtype=np.float32)
    base = {
        "xb": np.ascontiguousarray(xf.astype(ml_dtypes.bfloat16)),
        "wt": np.ascontiguousarray(W.T.astype(ml_dtypes.bfloat16)),
        "bias": np.asarray(b, np.float32).reshape(P, 1),
        "iota": np.tile(np.arange(P, dtype=ml_dtypes.bfloat16)[None, :],
                        (P, 1)),
    }
    in_maps = []
    for k in range(plan.n_cores):
        m = dict(base)
        for name, arr in plan.core_arrays[k].items():
            if name.startswith(("gidx", "crel")):
                s = name.split("_")[1]
                if plan.Csum[s] == 0:
                    continue
            m[name] = arr
        in_maps.append(m)
    return in_maps


def run(x, edge_index, n_nodes, W, b, trace=False, trace_cores=None):
    from concourse.bass_utils import run_bass_kernel_spmd

    x = np.asarray(x)
    edge_index = np.asarray(edge_index)
    W = np.asarray(W)
    b = np.asarray(b)
    n_nodes = int(n_nodes)
    row = edge_index[0].astype(np.int64)
    col = edge_index[1].astype(np.int64)

    plan = Plan(row, col, n_nodes)
    nc = build_program(plan, x.shape[1], W.shape[0])
    in_maps = make_in_maps(plan, x, W, b)
    res = run_bass_kernel_spmd(nc, in_maps, core_ids=list(range(plan.n_cores)),
                               trace=trace, trace_cores=trace_cores)
    out_f = W.shape[0]
    out = np.empty((n_nodes, out_f), np.float32)
    for k in range(plan.n_cores):
        o_k = np.asarray(res.results[k]["out"], np.float32)  # [out_f, W*P]
        for j, g in enumerate(plan.core_windows[k]):
            if g < 0:
                continue
            v = min(P, n_nodes - g * P)
            out[g * P:g * P + v] = o_k[:, j * P:j * P + v].T
    return np.ascontiguousarray(out), res


def kernel(x, edge_index, n_nodes, W, b):
    out, _ = run(x, edge_index, n_nodes, W, b)
    return out
